# revision 1
# baseline (speedup 1.0000x reference)
"""Trainium2 Bass kernel for the Clifford-algebra geometric product.

  out[..., j] = sum_{i,k} a[..., i] * cayley[i, j, k] * b[..., k]

Full inputs a, b: (2048, 1024, 8) fp32, cayley: (8, 8, 8) fp32.
Sharding: pure data parallelism over the leading batch axis across 8
NeuronCores; the cayley table's nonzero structure is folded into the
instruction stream (immediates), so it needs no on-device storage.

Per-core layout: the local (256*1024, 8) position-major array is viewed as
[128 partitions, 2048*8 free] (position-major split across partitions).
For each tile of W positions/partition:
  - 64 scalar_tensor_tensor ops compute signed products
      p0[:, j*8+l, :] = (a_i * sign) * b_k      for term l of output blade j
  - 3 full-width tensor_tensor adds do the binary-tree reduction; the last
    level writes with a transposed access pattern directly into the
    natural (position, blade) output tile.
"""

import sys

if "/opt/trn_rl_repo" not in sys.path:
    sys.path.insert(0, "/opt/trn_rl_repo")

import numpy as np

N_CORES = 8
P = 128  # SBUF partitions
N = 8    # blades
W = 256  # positions per partition per tile

_module_cache = {}


def _terms_by_j(cayley: np.ndarray):
    """Group the nonzero cayley entries by output blade j."""
    terms = [[] for _ in range(N)]
    for i in range(N):
        for j in range(N):
            for k in range(N):
                v = float(cayley[i, j, k])
                if v != 0.0:
                    terms[j].append((i, k, v))
    return terms


def _build_module(npos_local: int, terms):
    import concourse.bacc as bacc
    import concourse.mybir as mybir
    import concourse.tile as tile

    assert npos_local % (P * W) == 0
    T = npos_local // (P * W)
    fast = all(len(t) == 8 for t in terms)

    nc = bacc.Bacc(None, target_bir_lowering=False, debug=False)
    with tile.TileContext(nc) as tc:
        with tc.tile_pool(name="dram", bufs=1, space="DRAM") as dram:
            a = dram.tile((npos_local, N), mybir.dt.float32, kind="ExternalInput")
            b = dram.tile((npos_local, N), mybir.dt.float32, kind="ExternalInput")
            out = dram.tile((npos_local, N), mybir.dt.float32, kind="ExternalOutput")
            av = a[:].rearrange("(p f) n -> p (f n)", p=P)
            bv = b[:].rearrange("(p f) n -> p (f n)", p=P)
            ov = out[:].rearrange("(p f) n -> p (f n)", p=P)
            with (
                tc.tile_pool(name="io", bufs=2) as io_pool,
                tc.tile_pool(name="prod", bufs=1) as prod_pool,
            ):
                for t in range(T):
                    sl = slice(t * W * N, (t + 1) * W * N)
                    ta = io_pool.tile([P, W, N], mybir.dt.float32, tag="ta")
                    tb = io_pool.tile([P, W, N], mybir.dt.float32, tag="tb")
                    to = io_pool.tile([P, W, N], mybir.dt.float32, tag="to")
                    nc.sync.dma_start(
                        out=ta[:].rearrange("p f n -> p (f n)"), in_=av[:, sl]
                    )
                    nc.sync.dma_start(
                        out=tb[:].rearrange("p f n -> p (f n)"), in_=bv[:, sl]
                    )
                    if fast:
                        p0 = prod_pool.tile([P, 64, W], mybir.dt.float32, tag="p0")
                        p1 = prod_pool.tile([P, 32, W], mybir.dt.float32, tag="p1")
                        p2 = prod_pool.tile([P, 16, W], mybir.dt.float32, tag="p2")
                        for j in range(N):
                            for l, (i, k, v) in enumerate(terms[j]):
                                nc.vector.scalar_tensor_tensor(
                                    out=p0[:, j * 8 + l, :],
                                    in0=ta[:, :, i],
                                    scalar=v,
                                    in1=tb[:, :, k],
                                    op0=mybir.AluOpType.mult,
                                    op1=mybir.AluOpType.mult,
                                )
                        nc.vector.tensor_tensor(
                            out=p1[:], in0=p0[:, 0::2, :], in1=p0[:, 1::2, :],
                            op=mybir.AluOpType.add,
                        )
                        nc.vector.tensor_tensor(
                            out=p2[:], in0=p1[:, 0::2, :], in1=p1[:, 1::2, :],
                            op=mybir.AluOpType.add,
                        )
                        nc.vector.tensor_tensor(
                            out=to[:].transpose([0, 2, 1]),
                            in0=p2[:, 0::2, :], in1=p2[:, 1::2, :],
                            op=mybir.AluOpType.add,
                        )
                    else:
                        # generic fallback: per-j product + sequential adds
                        pa = prod_pool.tile([P, W], mybir.dt.float32, tag="pa")
                        acc = prod_pool.tile([P, W], mybir.dt.float32, tag="acc")
                        for j in range(N):
                            if not terms[j]:
                                nc.vector.memset(to[:, :, j], 0.0)
                                continue
                            i, k, v = terms[j][0]
                            nc.vector.scalar_tensor_tensor(
                                out=acc[:], in0=ta[:, :, i], scalar=v,
                                in1=tb[:, :, k],
                                op0=mybir.AluOpType.mult, op1=mybir.AluOpType.mult,
                            )
                            for (i, k, v) in terms[j][1:]:
                                nc.vector.scalar_tensor_tensor(
                                    out=pa[:], in0=ta[:, :, i], scalar=v,
                                    in1=tb[:, :, k],
                                    op0=mybir.AluOpType.mult, op1=mybir.AluOpType.mult,
                                )
                                nc.vector.tensor_tensor(
                                    out=acc[:], in0=acc[:], in1=pa[:],
                                    op=mybir.AluOpType.add,
                                )
                            nc.vector.tensor_copy(out=to[:, :, j], in_=acc[:])
                    nc.sync.dma_start(
                        out=ov[:, sl], in_=to[:].rearrange("p f n -> p (f n)")
                    )
    nc.compile()
    return nc, a.name, b.name, out.name


W_V2 = 256
GP_COLS = 0
TREE_GP_COLS = 0
USE_JOINT = True
RAGGED_WIDTHS = (128,)


def _dim_structures(size, max_digits=1):
    # With interleaved operands the inner free dim is (N, ncols) and walrus
    # limits these ops to partition + 2 free dims -> single-dim batches only.
    # With plane (deinterleaved) operands the inner dim is contiguous, so a
    # 2-digit batch dim is legal (3 free dims total... verifier allows 2-3).
    out = [(size,)]
    if max_digits >= 2:
        if size == 4:
            out = [(2, 2), (4,)]
        elif size == 6:
            out = [(2, 3), (3, 2), (6,)]
        elif size == 8:
            out = [(2, 4), (4, 2), (8,)]
    return out


def _enum_affine(counts, allowed):
    """Yield (offset, steps, addrs) where addrs = nested iteration of counts,
    all distinct, within `allowed` set."""
    import itertools

    nd = len(counts)
    for off in allowed:
        for steps in itertools.product(range(-7, 8), repeat=nd):
            if any(s == 0 for s in steps):
                continue
            addrs = []
            ok = True
            for digits in itertools.product(*[range(c) for c in counts]):
                a = off + sum(d * s for d, s in zip(digits, steps))
                if a < 0 or a > 7 or a not in allowed:
                    ok = False
                    break
                addrs.append(a)
            if ok and len(set(addrs)) == len(addrs):
                yield off, steps, addrs


def _image_affine(counts, kseq):
    """If kseq is affine w.r.t. digit structure `counts`, return (koff, ksteps)."""
    import itertools

    koff = kseq[0]
    ksteps = []
    stride = 1
    # compute strides of each digit position in the flattened order
    strides = []
    for c in reversed(counts):
        strides.insert(0, stride)
        stride *= c
    for d, c in enumerate(counts):
        if c > 1:
            ksteps.append(kseq[strides[d]] - koff)
        else:
            ksteps.append(0)
    for idx, digits in enumerate(itertools.product(*[range(c) for c in counts])):
        pred = koff + sum(dg * s for dg, s in zip(digits, ksteps))
        if kseq[idx] != pred:
            return None
    return koff, ksteps


def _decompose_class(i_set, pi_row, max_digits=1):
    """Greedy: cover i_set with affine batches whose pi-image is affine.
    Returns list of (counts, i_off, i_steps, k_off, k_steps) or None."""
    remaining = set(i_set)
    batches = []
    while remaining:
        n = len(remaining)
        found = None
        sizes = [s for s in (8, 7, 6, 5, 4, 3, 2) if s <= n and (n - s) % 2 == 0]
        for size in sizes:
            for counts in _dim_structures(size, max_digits=max_digits):
                for off, steps, addrs in _enum_affine(counts, remaining):
                    kseq = [pi_row[a] for a in addrs]
                    img = _image_affine(counts, kseq)
                    if img is not None:
                        found = (counts, off, steps, img[0], img[1])
                        break
                if found:
                    break
            if found:
                break
        if not found:
            return None
        counts, off, steps, koff, ksteps = found
        for digits_addr in _enum_affine(counts, remaining):
            pass  # not needed; recompute addrs directly
        # remove covered addrs
        import itertools

        for digits in itertools.product(*[range(c) for c in counts]):
            remaining.discard(off + sum(d * s for d, s in zip(digits, steps)))
        batches.append(found)
    return batches


def build_plan(cayley, max_digits=1):
    """Return per-j list of (sign, counts, i_off, i_steps, k_off, k_steps),
    or None if cayley doesn't fit the fast path."""
    pi = np.full((N, N), -1, dtype=int)
    sg = np.zeros((N, N), dtype=np.float64)
    for j in range(N):
        for i in range(N):
            ks = np.nonzero(cayley[i, j, :])[0]
            if len(ks) != 1:
                return None
            pi[j, i] = int(ks[0])
            sg[j, i] = float(cayley[i, j, ks[0]])
    if not np.all(np.abs(np.abs(sg) - 1.0) < 1e-12):
        return None
    plan = []
    for j in range(N):
        ops = []
        for sign in (1.0, -1.0):
            i_set = [int(i) for i in range(N) if sg[j, i] == sign]
            if not i_set:
                continue
            batches = _decompose_class(i_set, [int(x) for x in pi[j]], max_digits=max_digits)
            if batches is None:
                return None
            for (counts, ioff, isteps, koff, ksteps) in batches:
                ops.append((sign, counts, ioff, isteps, koff, ksteps))
        plan.append(ops)
    return plan


# ---------------- kernel build ----------------



def build_plan_joint(cayley):
    """Greedy cover of all 64 (i,j) product terms by arithmetic runs in the
    joint (i,j) lattice (k and the dest group g=i*8+j must also be
    arithmetic; sign uniform per run). Allows zero steps (broadcast reads).
    Returns [(sign, L, i0, di, k0, dk, j0, dj)] or None."""
    term = {}
    for i in range(N):
        for j in range(N):
            ks = np.nonzero(cayley[i, j, :])[0]
            if len(ks) != 1:
                return None
            v = float(cayley[i, j, ks[0]])
            if abs(abs(v) - 1.0) > 1e-12:
                return None
            term[(i, j)] = (int(ks[0]), v)
    remaining = set(term.keys())
    ops = []
    while remaining:
        best = None
        for L in (8, 6, 5, 4, 3, 2):
            if best:
                break
            for (i0, j0) in sorted(remaining):
                for di in range(-7, 8):
                    for dj in range(-7, 8):
                        if di == 0 and dj == 0:
                            continue
                        seq = [(i0 + m * di, j0 + m * dj) for m in range(L)]
                        if not all(
                            0 <= x < N and 0 <= y < N and (x, y) in remaining
                            for x, y in seq
                        ):
                            continue
                        ks = [term[xy][0] for xy in seq]
                        sg = [term[xy][1] for xy in seq]
                        if len(set(sg)) != 1:
                            continue
                        dk = ks[1] - ks[0]
                        if any(ks[m] != ks[0] + m * dk for m in range(L)):
                            continue
                        g = [x * 8 + y for x, y in seq]
                        dg = g[1] - g[0]
                        if any(g[m] != g[0] + m * dg for m in range(L)):
                            continue
                        best = (sg[0], L, i0, di, ks[0], dk, j0, dj)
                        break
                    if best:
                        break
                if best:
                    break
        if best is None:
            return None
        s, L, i0, di, k0, dk, j0, dj = best
        for m in range(L):
            remaining.discard((i0 + m * di, j0 + m * dj))
        ops.append(best)
    return ops

def build_module_planes(npos_local, plan, W=256, gp_cols=0, tree_gp_cols=0,
                        joint_plan=None, widths=None):
    """Deinterleave a,b into blade planes on ScalarE, then all products and
    tree adds are contiguous DVE/GPSIMD ops. L3 writes the interleaved
    output tile directly (strided dest).

    tree_gp_cols: the last `tree_gp_cols` position-columns of every tree
    level run on GPSIMD (3 big contiguous ops/tile), the rest on DVE.
    Products stay on DVE (GPSIMD's ~1.5us/op floor makes small ops lousy)."""
    import concourse.bacc as bacc
    import concourse.mybir as mybir
    import concourse.tile as tile
    from concourse.bass import AP

    if widths is None:
        assert npos_local % (P * W) == 0
        widths = [W] * (npos_local // (P * W))
    assert max(widths) <= W and sum(widths) * P == npos_local
    dve_cols = W - gp_cols

    nc = bacc.Bacc(None, target_bir_lowering=False, debug=False)
    with tile.TileContext(nc) as tc:
        with tc.tile_pool(name="dram", bufs=1, space="DRAM") as dram:
            a = dram.tile((npos_local, N), mybir.dt.float32, kind="ExternalInput")
            b = dram.tile((npos_local, N), mybir.dt.float32, kind="ExternalInput")
            out = dram.tile((npos_local, N), mybir.dt.float32, kind="ExternalOutput")
            av = a[:].rearrange("(p f) n -> p (f n)", p=P)
            bv = b[:].rearrange("(p f) n -> p (f n)", p=P)
            ov = out[:].rearrange("(p f) n -> p (f n)", p=P)

            streams = []
            if dve_cols > 0:
                streams.append(("dve", 0, dve_cols))
            if gp_cols > 0:
                streams.append(("gp", dve_cols, gp_cols))

            with (
                tc.tile_pool(name="io", bufs=2) as io_pool,
                tc.tile_pool(name="planes", bufs=1) as plane_pool,
                tc.tile_pool(name="prod", bufs=1) as prod_pool,
                tc.tile_pool(name="zeros", bufs=1) as zero_pool,
            ):
                # prewarm ScalarE's activation table (ACT_TABLE_LOAD ~2.6us)
                # before the first DMA lands so tile-0 deps don't pay for it
                warm = zero_pool.tile([P, 2], mybir.dt.float32, tag="warm")
                nc.vector.memset(warm[:, 0:1], 0.0)
                nc.scalar.copy(out=warm[:, 1:2], in_=warm[:, 0:1])
                gp_has_neg = gp_cols > 0 and any(
                    s < 0 for ops in plan for (s, *_r) in ops
                )
                if gp_has_neg:
                    zeros_gp = zero_pool.tile(
                        [P, N, gp_cols], mybir.dt.float32, tag="zgp"
                    )
                    nc.gpsimd.memset(zeros_gp[:], 0.0)
                pos0 = 0
                for t, wt in enumerate(widths):
                    sl = slice(pos0 * N, (pos0 + wt) * N)
                    pos0 += wt
                    streams_t = (
                        [("dve", 0, wt)] if gp_cols == 0 else streams
                    )
                    ta = io_pool.tile([P, wt, N], mybir.dt.float32, tag="ta")
                    tb = io_pool.tile([P, wt, N], mybir.dt.float32, tag="tb")
                    to = plane_pool.tile([P, wt, N], mybir.dt.float32, tag="to")
                    nc.sync.dma_start(
                        out=ta[:].rearrange("p f n -> p (f n)"), in_=av[:, sl]
                    )
                    nc.sync.dma_start(
                        out=tb[:].rearrange("p f n -> p (f n)"), in_=bv[:, sl]
                    )
                    taP = plane_pool.tile([P, N, wt], mybir.dt.float32, tag="taP")
                    tbP = plane_pool.tile([P, N, wt], mybir.dt.float32, tag="tbP")
                    toP = plane_pool.tile([P, N, wt], mybir.dt.float32, tag="toP")
                    # deinterleave: ScalarE in steady state; DVE for tile 0
                    # (DVE is otherwise idle during the pipeline fill, and this
                    # takes ScalarE off tile-0's critical path)
                    deint = nc.vector if t == 0 else nc.scalar
                    if t == 0:
                        deint.tensor_copy(out=taP[:], in_=ta[:].transpose([0, 2, 1]))
                        deint.tensor_copy(out=tbP[:], in_=tb[:].transpose([0, 2, 1]))
                    else:
                        nc.scalar.copy(out=taP[:], in_=ta[:].transpose([0, 2, 1]))
                        nc.scalar.copy(out=tbP[:], in_=tb[:].transpose([0, 2, 1]))
                    joint_has_neg = joint_plan is not None and any(
                        o[0] < 0 for o in joint_plan
                    )
                    if joint_has_neg:
                        # negated b planes on ScalarE -> negative-sign products
                        # become plain tensor_tensor (cheaper dispatch than STT)
                        bnP = plane_pool.tile(
                            [P, N, wt], mybir.dt.float32, tag="bnP"
                        )
                        nc.scalar.mul(out=bnP[:], in_=tbP[:], mul=-1.0)
                        bnP_b = bnP[:]
                    for sname, col0, ncols in streams_t:
                        eng = nc.vector if sname == "dve" else nc.gpsimd
                        is_gp = sname == "gp"
                        p0 = prod_pool.tile(
                            [P, 64, ncols], mybir.dt.float32, tag=f"p0{sname}"
                        )
                        p1 = prod_pool.tile(
                            [P, 32, ncols], mybir.dt.float32, tag=f"p1{sname}"
                        )
                        p2 = prod_pool.tile(
                            [P, 16, ncols], mybir.dt.float32, tag=f"p2{sname}"
                        )
                        if is_gp and gp_has_neg:
                            bnegP = prod_pool.tile(
                                [P, N, ncols], mybir.dt.float32, tag="bnegP"
                            )
                            nc.gpsimd.tensor_tensor(
                                out=bnegP[:],
                                in0=zeros_gp[:],
                                in1=tbP[:, :, col0 : col0 + ncols],
                                op=mybir.AluOpType.subtract,
                            )
                            bnegP_b = bnegP[:]
                        taP_b = taP[:]
                        tbP_b = tbP[:]
                        p0_b = p0[:]
                        pfree_pl = N * wt
                        pfree_p0 = 64 * ncols
                        if joint_plan is not None:
                            for (sgn, L, i0, di, k0, dk, j0, dj) in joint_plan:
                                in0 = AP(
                                    taP_b.tensor,
                                    taP_b.offset + i0 * wt + col0,
                                    [[pfree_pl, P], [di * wt, L], [1, ncols]],
                                )
                                in1 = AP(
                                    tbP_b.tensor,
                                    tbP_b.offset + k0 * wt + col0,
                                    [[pfree_pl, P], [dk * wt, L], [1, ncols]],
                                )
                                dst = AP(
                                    p0_b.tensor,
                                    p0_b.offset + (i0 * N + j0) * ncols,
                                    [[pfree_p0, P],
                                     [(di * N + dj) * ncols, L],
                                     [1, ncols]],
                                )
                                if sgn < 0:
                                    in1 = AP(
                                        bnP_b.tensor,
                                        bnP_b.offset + k0 * wt + col0,
                                        [[pfree_pl, P], [dk * wt, L], [1, ncols]],
                                    )
                                eng.tensor_tensor(
                                    out=dst, in0=in0, in1=in1,
                                    op=mybir.AluOpType.mult,
                                )
                        else:
                          for j in range(N):
                            for (sign, counts, ioff, isteps, koff, ksteps) in plan[j]:
                                dims_i = [[s * wt, c] for s, c in zip(isteps, counts)]
                                dims_k = [[s * wt, c] for s, c in zip(ksteps, counts)]
                                dims_kn = [
                                    [s * ncols, c] for s, c in zip(ksteps, counts)
                                ]
                                in0 = AP(
                                    taP_b.tensor,
                                    taP_b.offset + ioff * wt + col0,
                                    [[pfree_pl, P]] + dims_i + [[1, ncols]],
                                )
                                dst = AP(
                                    p0_b.tensor,
                                    p0_b.offset + (ioff * N + j) * ncols,
                                    [[pfree_p0, P]]
                                    + [[s * N * ncols, c] for s, c in zip(isteps, counts)]
                                    + [[1, ncols]],
                                )
                                if sign > 0 or not is_gp:
                                    in1 = AP(
                                        tbP_b.tensor,
                                        tbP_b.offset + koff * wt + col0,
                                        [[pfree_pl, P]] + dims_k + [[1, ncols]],
                                    )
                                    if sign > 0:
                                        eng.tensor_tensor(
                                            out=dst, in0=in0, in1=in1,
                                            op=mybir.AluOpType.mult,
                                        )
                                    else:
                                        eng.scalar_tensor_tensor(
                                            out=dst, in0=in0, scalar=-1.0, in1=in1,
                                            op0=mybir.AluOpType.mult,
                                            op1=mybir.AluOpType.mult,
                                        )
                                else:
                                    in1 = AP(
                                        bnegP_b.tensor,
                                        bnegP_b.offset + koff * ncols,
                                        [[N * ncols, P]] + dims_kn + [[1, ncols]],
                                    )
                                    eng.tensor_tensor(
                                        out=dst, in0=in0, in1=in1,
                                        op=mybir.AluOpType.mult,
                                    )
                        tg = min(tree_gp_cols, ncols) if sname == "dve" else 0
                        tranges = [(eng, 0, ncols - tg)]
                        if tg > 0:
                            tranges.append((nc.gpsimd, ncols - tg, tg))
                        for teng, tc0, tcn in tranges:
                            if tcn <= 0:
                                continue
                            teng.tensor_tensor(
                                out=p1[:, :, tc0 : tc0 + tcn],
                                in0=p0[:, 0:32, tc0 : tc0 + tcn],
                                in1=p0[:, 32:64, tc0 : tc0 + tcn],
                                op=mybir.AluOpType.add,
                            )
                            teng.tensor_tensor(
                                out=p2[:, :, tc0 : tc0 + tcn],
                                in0=p1[:, 0:16, tc0 : tc0 + tcn],
                                in1=p1[:, 16:32, tc0 : tc0 + tcn],
                                op=mybir.AluOpType.add,
                            )
                            teng.tensor_tensor(
                                out=toP[:, :, col0 + tc0 : col0 + tc0 + tcn],
                                in0=p2[:, 0:8, tc0 : tc0 + tcn],
                                in1=p2[:, 8:16, tc0 : tc0 + tcn],
                                op=mybir.AluOpType.add,
                            )
                        nc.scalar.copy(
                            out=to[:, col0 : col0 + ncols, :],
                            in_=toP[:, :, col0 : col0 + ncols].transpose([0, 2, 1]),
                        )
                    if t == len(widths) - 1 and wt % 2 == 0 and gp_cols == 0:
                        # last tile: store halves as they reinterleave so the
                        # tail drains sooner
                        h = wt // 2
                        nc.sync.dma_start(
                            out=ov[:, sl][:, : h * N],
                            in_=to[:, :h, :].rearrange("p f n -> p (f n)"),
                        )
                        nc.sync.dma_start(
                            out=ov[:, sl][:, h * N :],
                            in_=to[:, h:, :].rearrange("p f n -> p (f n)"),
                        )
                    else:
                        nc.sync.dma_start(
                            out=ov[:, sl], in_=to[:].rearrange("p f n -> p (f n)")
                        )
    nc.compile()
    return nc, a.name, b.name, out.name


def _get_module(npos_local: int, cayley: np.ndarray):
    key = (npos_local, cayley.tobytes())
    if key not in _module_cache:
        plan = build_plan(cayley, max_digits=1)
        jp = build_plan_joint(cayley) if USE_JOINT else None
        if jp is not None:
            # positives first: DVE can start them as soon as the deinterleaves
            # land, while ScalarE still computes the negated b-planes
            jp = sorted(jp, key=lambda o: o[0], reverse=True)
        if plan is not None and npos_local % (P * W_V2) == 0:
            ftot = npos_local // P
            wl = RAGGED_WIDTHS
            widths = (
                list(wl) + [W_V2] * ((ftot - sum(wl) - sum(wl)) // W_V2)
                + list(reversed(wl))
                if sum(wl) * 2 <= ftot
                and (ftot - 2 * sum(wl)) % W_V2 == 0
                else None
            )
            _module_cache[key] = build_module_planes(
                npos_local, plan, W=W_V2, gp_cols=GP_COLS,
                tree_gp_cols=TREE_GP_COLS, joint_plan=jp, widths=widths
            )
        else:
            _module_cache[key] = _build_module(npos_local, _terms_by_j(cayley))
    return _module_cache[key]


def _run(inputs: dict, trace: bool = False, tmpdir=None):
    a = np.asarray(inputs["a"], dtype=np.float32)
    b = np.asarray(inputs["b"], dtype=np.float32)
    cayley = np.asarray(inputs["cayley"], dtype=np.float32)
    B, S, NN = a.shape
    assert NN == N and b.shape == a.shape and cayley.shape == (N, N, N)
    assert B % N_CORES == 0
    nb = B // N_CORES
    npos_local = nb * S

    nc, a_name, b_name, out_name = _get_module(npos_local, cayley)

    a_sh = a.reshape(N_CORES, npos_local, N)
    b_sh = b.reshape(N_CORES, npos_local, N)
    in_maps = [
        {a_name: np.ascontiguousarray(a_sh[c]), b_name: np.ascontiguousarray(b_sh[c])}
        for c in range(N_CORES)
    ]

    from concourse import bass_utils

    kwargs = {}
    if trace:
        _install_ntff_shim()
        bass_utils.upload_artifacts = lambda d: f"local:{d}"
        kwargs = {"trace": True, "tmpdir": tmpdir}
    res = bass_utils.run_bass_kernel_spmd(
        nc, in_maps, core_ids=list(range(N_CORES)), **kwargs
    )
    out = np.concatenate(
        [res.results[c][out_name].reshape(1, nb, S, N) for c in range(N_CORES)], axis=0
    ).reshape(B, S, N)
    return out, res


def kernel(**inputs) -> np.ndarray:
    out, _ = _run(inputs, trace=False)
    return out


def kernel_traced(**inputs):
    """Run with NTFF profiling; returns (out, exec_time_ns, trace_path)."""
    import tempfile

    out, res = _run(inputs, trace=True, tmpdir=tempfile.mkdtemp(prefix="gp_trace_"))
    trace_path = res.instructions_and_trace[1] if res.instructions_and_trace else None
    return out, res.exec_time_ns, trace_path


def _install_ntff_shim():
    """Provide antenv.axon_hooks with an NTFF profile hook if missing."""
    try:
        from antenv.axon_hooks import get_axon_ntff_profile_hook  # noqa: F401

        return
    except ImportError:
        pass
    import types, ctypes, contextlib

    holder = {"hook": None}
    mod = types.ModuleType("antenv.axon_hooks")
    mod.set_axon_ntff_profile_hook = lambda h: holder.__setitem__("hook", h)
    mod.get_axon_ntff_profile_hook = lambda: holder["hook"]
    sys.modules["antenv.axon_hooks"] = mod

    so_path = "/opt/axon/libaxon_pjrt.so"
    try:
        lib = ctypes.CDLL(so_path)
        if not hasattr(lib, "axon_start_nrt_profile"):
            return
    except OSError:
        return
    lib.axon_start_nrt_profile.argtypes = [
        ctypes.POINTER(ctypes.c_int64),
        ctypes.c_size_t,
    ]
    lib.axon_start_nrt_profile.restype = ctypes.c_int64
    lib.axon_stop_nrt_profile.argtypes = [ctypes.c_char_p]
    lib.axon_stop_nrt_profile.restype = ctypes.c_int64

    @contextlib.contextmanager
    def _hook(output_dir, device_ids):
        import jax

        jax.devices()
        if device_ids:
            ids = (ctypes.c_int64 * len(device_ids))(*device_ids)
            rc = lib.axon_start_nrt_profile(ids, len(device_ids))
        else:
            rc = lib.axon_start_nrt_profile(None, 0)
        if rc != 0:
            raise RuntimeError(f"axon_start_nrt_profile rc={rc}")
        try:
            yield
        finally:
            n = lib.axon_stop_nrt_profile(str(output_dir).encode())
            print(f"profile: {n} file(s) written to {output_dir}", file=sys.stderr)

    mod.set_axon_ntff_profile_hook(_hook)



# revision 4
# speedup vs baseline: 2.1543x; 2.1543x over previous
"""Trainium2 Bass kernel for the Clifford-algebra geometric product.

  out[..., j] = sum_{i,k} a[..., i] * cayley[i, j, k] * b[..., k]

Full inputs a, b: (2048, 1024, 8) fp32, cayley: (8, 8, 8) fp32.
Sharding: pure data parallelism over the leading batch axis across 8
NeuronCores.

Fast path ("pauli"): Cl(3,0) is isomorphic to the 2x2 complex matrix
algebra M2(C) (Pauli matrices).  Writing each multivector as
  M = [[ (a0+a3) + i(a12+a123), (a1-a13) + i(a23-a2) ],
       [ (a1+a13) + i(a2+a23),  (a0-a3) + i(a123-a12) ]]
the geometric product is the 2x2 complex matmul C = A*B, and the output
coefficients are (sum/difference pairs of C entries)/2.  This cuts the
elementwise work from 120 ops/position (64 products + 56 tree adds) to
80 ops/position (16 transform + 32 products + 24 combine + 8 out), all
expressible as plain tensor_tensor adds/mults.

All compute runs in fp16 on the DVE at 2 elem/lane/cycle (the 2x_1p
packed mode; fp32 TT runs at 1x).  ScalarE (ACT) does the
interleaved<->plane transposes with the fp32<->fp16 conversion and the
0.5 scale folded in (transposed ACT copies cost the same with or
without scale/convert).  Measured end-to-end fp16 error vs the fp32
reference: ~8e-4 max-rel (gate is 2e-2).

Layout per tile of width w positions/partition:
  ta/tb  [P, w, 8] fp32   interleaved (contiguous DMA)
  tAB    [P, 16, w] fp16  blade planes: [0:8] = a*0.5, [8:16] = b
  tfAB   [P, 16, w] fp16  transformed planes, alpha(r,s,e)=4r+2s+e
                          (A in [0:8]) / beta(s,c,e)=4s+2c+e (B in [8:16])
  tp     [P, 32, w] fp16  products pi(r,s,e1,c,e2)=16r+8s+4e1+2c+e2
  tl     [P, 16, w] fp16  mu(r,s,h,c)=8r+4s+2h+c   (h=0 real, 1 imag)
  tC     [P, 8, w]  fp16  chi(r,e,c)=4r+2e+c
  toP    [P, 8, w]  fp16  output blade planes
  to     [P, w, 8]  fp32  interleaved output (ACT reinterleave)
"""

import sys

if "/opt/trn_rl_repo" not in sys.path:
    sys.path.insert(0, "/opt/trn_rl_repo")

import numpy as np

N_CORES = 8
P = 128  # SBUF partitions
N = 8    # blades

WIDTHS = (128, 384, 512, 512, 384, 128)  # sums to 2048 = npos_local / P

_module_cache = {}


def _canonical_cayley() -> np.ndarray:
    """Cl(3,0) geometric-product table, short-lex blade order (= reference)."""
    import itertools, functools, operator

    metric = [1, 1, 1]
    nv = len(metric)
    n = 2 ** nv
    basis = [1 << k for k in range(nv)]
    combos = itertools.chain.from_iterable(
        itertools.combinations(basis, r) for r in range(nv + 1))
    i2b = [functools.reduce(operator.or_, t, 0) for t in combos]
    b2i = {b: i for i, b in enumerate(i2b)}
    c = np.zeros((n, n, n), dtype=np.float32)
    for i, bi in enumerate(i2b):
        for j, bj in enumerate(i2b):
            a = bi >> 1
            s = 0
            while a:
                s += bin(a & bj).count("1")
                a >>= 1
            sign = -1.0 if (s & 1) else 1.0
            common = bi & bj
            k = 0
            while common:
                if common & 1:
                    sign *= metric[k]
                k += 1
                common >>= 1
            c[i, b2i[bi ^ bj], j] = sign
    return c


# ---------------- pauli fast path ----------------


def build_module_pauli(npos_local: int, widths):
    import concourse.bacc as bacc
    import concourse.mybir as mybir
    import concourse.tile as tile
    from concourse.bass import AP

    assert sum(widths) * P == npos_local
    f16 = mybir.dt.float16
    f32 = mybir.dt.float32
    ADD = mybir.AluOpType.add
    SUB = mybir.AluOpType.subtract
    MUL = mybir.AluOpType.mult

    nc = bacc.Bacc(None, target_bir_lowering=False, debug=False)
    with tile.TileContext(nc) as tc:
        with tc.tile_pool(name="dram", bufs=1, space="DRAM") as dram:
            a = dram.tile((npos_local, N), f32, kind="ExternalInput")
            b = dram.tile((npos_local, N), f32, kind="ExternalInput")
            out = dram.tile((npos_local, N), f32, kind="ExternalOutput")
            av = a[:].rearrange("(p f) n -> p (f n)", p=P)
            bv = b[:].rearrange("(p f) n -> p (f n)", p=P)
            ov = out[:].rearrange("(p f) n -> p (f n)", p=P)

            with (
                tc.tile_pool(name="io", bufs=2) as io_pool,
                tc.tile_pool(name="pl", bufs=1) as pl_pool,
                tc.tile_pool(name="oP", bufs=2) as oP_pool,
                tc.tile_pool(name="wrm", bufs=1) as wrm_pool,
            ):
                # prewarm ScalarE's activation table (ACT_TABLE_LOAD ~2.6us)
                # before the first DMA lands so tile-0 deps don't pay for it
                warm = wrm_pool.tile([P, 2], f32, tag="warm")
                nc.vector.memset(warm[:, 0:1], 0.0)
                nc.scalar.copy(out=warm[:, 1:2], in_=warm[:, 0:1])

                def ap(t, pfree, off, dims):
                    base = t[:]
                    return AP(base.tensor, base.offset + off,
                              [[pfree, P]] + dims)

                pos0 = 0
                for t, w in enumerate(widths):
                    sl = slice(pos0 * N, (pos0 + w) * N)
                    pos0 += w
                    ta = io_pool.tile([P, w, N], f32, tag="ta")
                    tb = io_pool.tile([P, w, N], f32, tag="tb")
                    to = io_pool.tile([P, w, N], f32, tag="to")
                    tAB = pl_pool.tile([P, 16, w], f16, tag="tAB")
                    tfAB = pl_pool.tile([P, 16, w], f16, tag="tfAB")
                    tp = pl_pool.tile([P, 32, w], f16, tag="tp")
                    tl = pl_pool.tile([P, 16, w], f16, tag="tl")
                    tC = pl_pool.tile([P, 8, w], f16, tag="tC")
                    toP = oP_pool.tile([P, 8, w], f16, tag="toP")

                    nc.sync.dma_start(
                        out=ta[:].rearrange("p f n -> p (f n)"), in_=av[:, sl])
                    nc.sync.dma_start(
                        out=tb[:].rearrange("p f n -> p (f n)"), in_=bv[:, sl])

                    # deinterleave + fp16 convert on ScalarE; 0.5 folded into a
                    nc.scalar.mul(out=tAB[:, 0:8, :],
                                  in_=ta[:].transpose([0, 2, 1]), mul=0.5)
                    nc.scalar.copy(out=tAB[:, 8:16, :],
                                   in_=tb[:].transpose([0, 2, 1]))

                    # --- TF: 4 DVE ops [P,2,2,w] over (operand, pair, w) ---
                    # gamma: 0=M00r 1=M00i 2=M01r 3=M01i 4=M10r 5=M10i 6=M11r 7=M11i
                    pf = 16 * w
                    for (oo, so, i0, s0, i1, s1, alu) in (
                        (0 * w, 1 * w, 0 * w, 4 * w, 3 * w, 4 * w, ADD),   # g0=m0+m3 g1=m4+m7
                        (4 * w, 1 * w, 1 * w, 1 * w, 5 * w, 1 * w, ADD),   # g4=m1+m5 g5=m2+m6
                        (2 * w, 4 * w, 1 * w, -1 * w, 5 * w, -2 * w, SUB),  # g2=m1-m5 g6=m0-m3
                        (3 * w, 4 * w, 6 * w, 1 * w, 2 * w, 2 * w, SUB),   # g3=m6-m2 g7=m7-m4
                    ):
                        nc.vector.tensor_tensor(
                            out=ap(tfAB, pf, oo,
                                   [[8 * w, 2], [so, 2], [1, w]]),
                            in0=ap(tAB, pf, i0,
                                   [[8 * w, 2], [s0, 2], [1, w]]),
                            in1=ap(tAB, pf, i1,
                                   [[8 * w, 2], [s1, 2], [1, w]]),
                            op=alu)

                    # --- PROD: 4 DVE ops [P,2,4,w] over (e1, (c,e2), w) ---
                    for r in (0, 1):
                        for s in (0, 1):
                            nc.vector.tensor_tensor(
                                out=ap(tp, 32 * w, (16 * r + 8 * s) * w,
                                       [[4 * w, 2], [w, 4], [1, w]]),
                                in0=ap(tfAB, pf, (4 * r + 2 * s) * w,
                                       [[w, 2], [0, 4], [1, w]]),
                                in1=ap(tfAB, pf, (8 + 4 * s) * w,
                                       [[0, 2], [w, 4], [1, w]]),
                                op=MUL)

                    # --- L1: 2 DVE ops [P,4,2,w] over ((r,s), c, w) ---
                    nc.vector.tensor_tensor(  # real: p(..00) - p(..11)
                        out=ap(tl, 16 * w, 0,
                               [[4 * w, 4], [w, 2], [1, w]]),
                        in0=ap(tp, 32 * w, 0,
                               [[8 * w, 4], [2 * w, 2], [1, w]]),
                        in1=ap(tp, 32 * w, 5 * w,
                               [[8 * w, 4], [2 * w, 2], [1, w]]),
                        op=SUB)
                    nc.vector.tensor_tensor(  # imag: p(..01) + p(..10)
                        out=ap(tl, 16 * w, 2 * w,
                               [[4 * w, 4], [w, 2], [1, w]]),
                        in0=ap(tp, 32 * w, 1 * w,
                               [[8 * w, 4], [2 * w, 2], [1, w]]),
                        in1=ap(tp, 32 * w, 4 * w,
                               [[8 * w, 4], [2 * w, 2], [1, w]]),
                        op=ADD)

                    # --- L2: 1 DVE op [P,2,4,w] over (r, (e,c), w) ---
                    nc.vector.tensor_tensor(
                        out=ap(tC, 8 * w, 0, [[4 * w, 2], [w, 4], [1, w]]),
                        in0=ap(tl, 16 * w, 0, [[8 * w, 2], [w, 4], [1, w]]),
                        in1=ap(tl, 16 * w, 4 * w,
                               [[8 * w, 2], [w, 4], [1, w]]),
                        op=ADD)

                    # --- OTF: 3 DVE ops -> blade planes ---
                    # o0=C0+C5 o1=C1+C4 o6=C6+C3 o7=C7+C2
                    nc.vector.tensor_tensor(
                        out=ap(toP, 8 * w, 0, [[6 * w, 2], [w, 2], [1, w]]),
                        in0=ap(tC, 8 * w, 0, [[6 * w, 2], [w, 2], [1, w]]),
                        in1=ap(tC, 8 * w, 5 * w,
                               [[-2 * w, 2], [-w, 2], [1, w]]),
                        op=ADD)
                    # o2=C6-C3 o3=C0-C5
                    nc.vector.tensor_tensor(
                        out=ap(toP, 8 * w, 2 * w, [[w, 2], [1, w]]),
                        in0=ap(tC, 8 * w, 6 * w, [[-6 * w, 2], [1, w]]),
                        in1=ap(tC, 8 * w, 3 * w, [[2 * w, 2], [1, w]]),
                        op=SUB)
                    # o4=C2-C7 o5=C4-C1
                    nc.vector.tensor_tensor(
                        out=ap(toP, 8 * w, 4 * w, [[w, 2], [1, w]]),
                        in0=ap(tC, 8 * w, 2 * w, [[2 * w, 2], [1, w]]),
                        in1=ap(tC, 8 * w, 7 * w, [[-6 * w, 2], [1, w]]),
                        op=SUB)

                    # reinterleave + fp32 convert on ScalarE
                    nc.scalar.copy(out=to[:], in_=toP[:].transpose([0, 2, 1]))

                    if t == len(widths) - 1 and w % 2 == 0:
                        # drain the tail sooner: store halves
                        h = w // 2
                        nc.sync.dma_start(
                            out=ov[:, sl][:, : h * N],
                            in_=to[:, :h, :].rearrange("p f n -> p (f n)"))
                        nc.sync.dma_start(
                            out=ov[:, sl][:, h * N:],
                            in_=to[:, h:, :].rearrange("p f n -> p (f n)"))
                    else:
                        nc.sync.dma_start(
                            out=ov[:, sl],
                            in_=to[:].rearrange("p f n -> p (f n)"))
    nc.compile()
    return nc, a.name, b.name, out.name


# ---------------- generic fallback (any cayley) ----------------


def _terms_by_j(cayley: np.ndarray):
    terms = [[] for _ in range(N)]
    for i in range(N):
        for j in range(N):
            for k in range(N):
                v = float(cayley[i, j, k])
                if v != 0.0:
                    terms[j].append((i, k, v))
    return terms


def _build_module(npos_local: int, terms):
    import concourse.bacc as bacc
    import concourse.mybir as mybir
    import concourse.tile as tile

    W = 256
    assert npos_local % (P * W) == 0
    T = npos_local // (P * W)
    fast = all(len(t) == 8 for t in terms)

    nc = bacc.Bacc(None, target_bir_lowering=False, debug=False)
    with tile.TileContext(nc) as tc:
        with tc.tile_pool(name="dram", bufs=1, space="DRAM") as dram:
            a = dram.tile((npos_local, N), mybir.dt.float32, kind="ExternalInput")
            b = dram.tile((npos_local, N), mybir.dt.float32, kind="ExternalInput")
            out = dram.tile((npos_local, N), mybir.dt.float32, kind="ExternalOutput")
            av = a[:].rearrange("(p f) n -> p (f n)", p=P)
            bv = b[:].rearrange("(p f) n -> p (f n)", p=P)
            ov = out[:].rearrange("(p f) n -> p (f n)", p=P)
            with (
                tc.tile_pool(name="io", bufs=2) as io_pool,
                tc.tile_pool(name="prod", bufs=1) as prod_pool,
            ):
                for t in range(T):
                    sl = slice(t * W * N, (t + 1) * W * N)
                    ta = io_pool.tile([P, W, N], mybir.dt.float32, tag="ta")
                    tb = io_pool.tile([P, W, N], mybir.dt.float32, tag="tb")
                    to = io_pool.tile([P, W, N], mybir.dt.float32, tag="to")
                    nc.sync.dma_start(
                        out=ta[:].rearrange("p f n -> p (f n)"), in_=av[:, sl]
                    )
                    nc.sync.dma_start(
                        out=tb[:].rearrange("p f n -> p (f n)"), in_=bv[:, sl]
                    )
                    if fast:
                        p0 = prod_pool.tile([P, 64, W], mybir.dt.float32, tag="p0")
                        p1 = prod_pool.tile([P, 32, W], mybir.dt.float32, tag="p1")
                        p2 = prod_pool.tile([P, 16, W], mybir.dt.float32, tag="p2")
                        for j in range(N):
                            for l, (i, k, v) in enumerate(terms[j]):
                                nc.vector.scalar_tensor_tensor(
                                    out=p0[:, j * 8 + l, :],
                                    in0=ta[:, :, i],
                                    scalar=v,
                                    in1=tb[:, :, k],
                                    op0=mybir.AluOpType.mult,
                                    op1=mybir.AluOpType.mult,
                                )
                        nc.vector.tensor_tensor(
                            out=p1[:], in0=p0[:, 0::2, :], in1=p0[:, 1::2, :],
                            op=mybir.AluOpType.add,
                        )
                        nc.vector.tensor_tensor(
                            out=p2[:], in0=p1[:, 0::2, :], in1=p1[:, 1::2, :],
                            op=mybir.AluOpType.add,
                        )
                        nc.vector.tensor_tensor(
                            out=to[:].transpose([0, 2, 1]),
                            in0=p2[:, 0::2, :], in1=p2[:, 1::2, :],
                            op=mybir.AluOpType.add,
                        )
                    else:
                        pa = prod_pool.tile([P, W], mybir.dt.float32, tag="pa")
                        acc = prod_pool.tile([P, W], mybir.dt.float32, tag="acc")
                        for j in range(N):
                            if not terms[j]:
                                nc.vector.memset(to[:, :, j], 0.0)
                                continue
                            i, k, v = terms[j][0]
                            nc.vector.scalar_tensor_tensor(
                                out=acc[:], in0=ta[:, :, i], scalar=v,
                                in1=tb[:, :, k],
                                op0=mybir.AluOpType.mult, op1=mybir.AluOpType.mult,
                            )
                            for (i, k, v) in terms[j][1:]:
                                nc.vector.scalar_tensor_tensor(
                                    out=pa[:], in0=ta[:, :, i], scalar=v,
                                    in1=tb[:, :, k],
                                    op0=mybir.AluOpType.mult, op1=mybir.AluOpType.mult,
                                )
                                nc.vector.tensor_tensor(
                                    out=acc[:], in0=acc[:], in1=pa[:],
                                    op=mybir.AluOpType.add,
                                )
                            nc.vector.tensor_copy(out=to[:, :, j], in_=acc[:])
                    nc.sync.dma_start(
                        out=ov[:, sl], in_=to[:].rearrange("p f n -> p (f n)")
                    )
    nc.compile()
    return nc, a.name, b.name, out.name


def _get_module(npos_local: int, cayley: np.ndarray):
    key = (npos_local, cayley.tobytes())
    if key not in _module_cache:
        if (npos_local % P == 0 and sum(WIDTHS) * P == npos_local
                and np.array_equal(cayley, _canonical_cayley())):
            _module_cache[key] = build_module_pauli(npos_local, WIDTHS)
        else:
            _module_cache[key] = _build_module(npos_local, _terms_by_j(cayley))
    return _module_cache[key]


def _run(inputs: dict, trace: bool = False, tmpdir=None):
    a = np.asarray(inputs["a"], dtype=np.float32)
    b = np.asarray(inputs["b"], dtype=np.float32)
    cayley = np.asarray(inputs["cayley"], dtype=np.float32)
    B, S, NN = a.shape
    assert NN == N and b.shape == a.shape and cayley.shape == (N, N, N)
    assert B % N_CORES == 0
    nb = B // N_CORES
    npos_local = nb * S

    nc, a_name, b_name, out_name = _get_module(npos_local, cayley)

    a_sh = a.reshape(N_CORES, npos_local, N)
    b_sh = b.reshape(N_CORES, npos_local, N)
    in_maps = [
        {a_name: np.ascontiguousarray(a_sh[c]), b_name: np.ascontiguousarray(b_sh[c])}
        for c in range(N_CORES)
    ]

    from concourse import bass_utils

    kwargs = {}
    if trace:
        _install_ntff_shim()
        bass_utils.upload_artifacts = lambda d: f"local:{d}"
        kwargs = {"trace": True, "tmpdir": tmpdir}
    res = bass_utils.run_bass_kernel_spmd(
        nc, in_maps, core_ids=list(range(N_CORES)), **kwargs
    )
    out = np.concatenate(
        [res.results[c][out_name].reshape(1, nb, S, N) for c in range(N_CORES)], axis=0
    ).reshape(B, S, N)
    return out, res


def kernel(**inputs) -> np.ndarray:
    out, _ = _run(inputs, trace=False)
    return out


def kernel_traced(**inputs):
    """Run with NTFF profiling; returns (out, exec_time_ns, trace_path)."""
    import tempfile

    out, res = _run(inputs, trace=True, tmpdir=tempfile.mkdtemp(prefix="gp_trace_"))
    trace_path = res.instructions_and_trace[1] if res.instructions_and_trace else None
    return out, res.exec_time_ns, trace_path


def _install_ntff_shim():
    """Provide antenv.axon_hooks with an NTFF profile hook if missing."""
    try:
        from antenv.axon_hooks import get_axon_ntff_profile_hook  # noqa: F401

        return
    except ImportError:
        pass
    import types, ctypes, contextlib

    holder = {"hook": None}
    mod = types.ModuleType("antenv.axon_hooks")
    mod.set_axon_ntff_profile_hook = lambda h: holder.__setitem__("hook", h)
    mod.get_axon_ntff_profile_hook = lambda: holder["hook"]
    sys.modules["antenv.axon_hooks"] = mod

    so_path = "/opt/axon/libaxon_pjrt.so"
    try:
        lib = ctypes.CDLL(so_path)
        if not hasattr(lib, "axon_start_nrt_profile"):
            return
    except OSError:
        return
    lib.axon_start_nrt_profile.argtypes = [
        ctypes.POINTER(ctypes.c_int64),
        ctypes.c_size_t,
    ]
    lib.axon_start_nrt_profile.restype = ctypes.c_int64
    lib.axon_stop_nrt_profile.argtypes = [ctypes.c_char_p]
    lib.axon_stop_nrt_profile.restype = ctypes.c_int64

    @contextlib.contextmanager
    def _hook(output_dir, device_ids):
        import jax

        jax.devices()
        if device_ids:
            ids = (ctypes.c_int64 * len(device_ids))(*device_ids)
            rc = lib.axon_start_nrt_profile(ids, len(device_ids))
        else:
            rc = lib.axon_start_nrt_profile(None, 0)
        if rc != 0:
            raise RuntimeError(f"axon_start_nrt_profile rc={rc}")
        try:
            yield
        finally:
            n = lib.axon_stop_nrt_profile(str(output_dir).encode())
            print(f"profile: {n} file(s) written to {output_dir}", file=sys.stderr)

    mod.set_axon_ntff_profile_hook(_hook)
